# revision 1
# baseline (speedup 1.0000x reference)
"""CenterLoss kernel for Trainium2 (Bass/Tile), data-parallel over 8 NeuronCores.

reference:
    d_i = ||x_i||^2 + ||centers[l_i]||^2 - 2 x_i . centers[l_i]   (= ||x_i - c_{l_i}||^2)
    loss = mean_i clip(d_i, 1e-12, 1e12)

Only the label-gathered entry of the [N, C] distance matrix is used, so the
kernel never forms it: each core gathers centers[labels] with the Q7
dma_gather extended instruction (2048 rows per instruction), computes
(x - c)^2 via DVE subtract + ACT square-with-accumulate, reduces to a scalar
partial sum, and the host combines the 8 partials into the mean.
The clip is a provable no-op for this input distribution (d_i ~ chi^2-like,
concentrated around 256; min over N is >> 1e-12).

Sharding: x/labels split into 8 contiguous row shards; centers replicated.

Layouts per core (ROWS=8192 rows, D=128):
  x tile, chunk c: [128, 16*128] f32, partition p holds rows c*2048 + p*16 .. +15
                   (8 KiB contiguous per partition -> efficient DMA)
  gather, chunk c: dma_gather dst[i%128, i//128, :] = centers[idx_i], so host
                   orders idx_i = labels[c*2048 + (i%128)*16 + (i//128)] to
                   match the x layout. Indices int16, wrapped over 16
                   partitions: idxs[i%16, c*128 + i//16].
"""

import numpy as np

import concourse.bacc as bacc
import concourse.bass as bass
import concourse.tile as tile
from concourse import mybir
from concourse.bass_utils import run_bass_kernel_spmd
from concourse.library_config import mlp

N, C, D = 65536, 1000, 128
N_CORES = 8
P = 128
ROWS_PER_CORE = N // N_CORES            # 8192
CHUNK_ROWS = 512                        # rows gathered/processed per chunk
NCHUNK = ROWS_PER_CORE // CHUNK_ROWS    # 16
SUB = CHUNK_ROWS // P                   # 16 rows per partition per chunk
IDXCOLS = CHUNK_ROWS // 16              # 128 idx columns per chunk

_NC = None


def _build_nc():
    f32 = mybir.dt.float32
    nc = bacc.Bacc(trn_type="TRN2", num_swdge_queues=4, dynamic_dma_scratch_size=65536)

    x = nc.dram_tensor("x", [ROWS_PER_CORE, D], f32, kind="ExternalInput")
    idx16 = nc.dram_tensor(
        "idx16", [P, NCHUNK * IDXCOLS], mybir.dt.int16, kind="ExternalInput"
    )
    centers = nc.dram_tensor("centers", [C, D], f32, kind="ExternalInput")
    out = nc.dram_tensor("out", [1, 1], f32, kind="ExternalOutput")

    # [NCHUNK, P, SUB*D]; partition p of chunk c holds rows c*2048 + p*16 .. +15
    x_r = x.ap().rearrange("(c p s) d -> c p (s d)", p=P, s=SUB)

    with tile.TileContext(nc) as tc:
        with (
            tc.tile_pool(name="xp", bufs=16) as xp,
            tc.tile_pool(name="cp", bufs=16) as cp,
            tc.tile_pool(name="small", bufs=1) as small,
            tc.tile_pool(name="psp", bufs=1, space="PSUM") as psp,
        ):
            # eager Q7 library load so the first gather doesn't stall on the
            # lazy IRAM code fetch
            nc.gpsimd.load_library(mlp)

            idx = small.tile([P, NCHUNK * IDXCOLS], mybir.dt.int16)
            nc.sync.dma_start(out=idx[:], in_=idx16.ap())

            acc = small.tile([P, NCHUNK], f32)
            # queues 1-3 generate descriptors on background Q7 workers; queue 0
            # generates inline on the Pool engine (a 4th worker) while the
            # background queues churn. Small chunks start data drains early.
            # queues 1-3 run on background Q7 workers; queue 0 generates inline
            # on the engine. Each period: 6 background enqueues, then 2 inline
            # gens (workers churn while the engine generates). The period of 8
            # matches the 8 DMASW sem lanes so lanes stay queue-consistent.
            QUEUE = [1, 2, 3, 0] * 4
            xts, cts = {}, {}
            for c in range(NCHUNK):
                xt = xp.tile([P, SUB * D], f32, tag="xt")
                nc.sync.dma_start(out=xt[:], in_=x_r[c])
                ct = cp.tile([P, SUB * D], f32, tag="ct")
                nc.gpsimd.dma_gather(
                    ct[:].rearrange("p (s d) -> p s d", s=SUB),
                    centers.ap(),
                    idx[:, c * IDXCOLS:(c + 1) * IDXCOLS],
                    CHUNK_ROWS,
                    CHUNK_ROWS,
                    D,
                    queue_num=QUEUE[c],
                    single_packet=False,
                )
                xts[c], cts[c] = xt, ct
            for c in range(NCHUNK):
                xt, ct = xts[c], cts[c]
                nc.vector.tensor_tensor(
                    out=xt[:], in0=xt[:], in1=ct[:], op=mybir.AluOpType.subtract
                )
                nc.scalar.activation(
                    out=xt[:],
                    in_=xt[:],
                    func=mybir.ActivationFunctionType.Square,
                    accum_out=acc[:, c:c + 1],
                )

            dsum = small.tile([P, 1], f32)
            nc.vector.tensor_reduce(
                out=dsum[:], in_=acc[:], axis=mybir.AxisListType.X,
                op=mybir.AluOpType.add,
            )
            ones = small.tile([P, 1], f32)
            nc.vector.memset(ones[:], 1.0)
            ps = psp.tile([1, 1], f32)
            nc.tensor.matmul(out=ps[:], lhsT=ones[:], rhs=dsum[:], start=True, stop=True)
            res = small.tile([1, 1], f32)
            nc.vector.tensor_copy(out=res[:], in_=ps[:])
            nc.sync.dma_start(out=out.ap(), in_=res[:])

    nc.compile()
    return nc


def _get_nc():
    global _NC
    if _NC is None:
        _NC = _build_nc()
    return _NC


def _make_idx16(lab_core):
    """Wrap one core's labels into the dma_gather int16 index layout."""
    idx16 = np.zeros((16, NCHUNK * IDXCOLS), dtype=np.int16)
    i = np.arange(CHUNK_ROWS)
    for c in range(NCHUNK):
        vals = lab_core[c * CHUNK_ROWS + (i % P) * SUB + (i // P)]
        idx16[i % 16, c * IDXCOLS + i // 16] = vals.astype(np.int16)
    # the 8 Q7 cores each read their own 16-partition replica of the indices
    return np.ascontiguousarray(np.tile(idx16, (8, 1)))


def make_in_maps(x, labels, centers):
    x = np.ascontiguousarray(np.asarray(x), dtype=np.float32)
    labels_np = np.asarray(labels).astype(np.int64)
    centers = np.ascontiguousarray(np.asarray(centers), dtype=np.float32)
    in_maps = []
    for m in range(N_CORES):
        lo = m * ROWS_PER_CORE
        in_maps.append({
            "x": x[lo:lo + ROWS_PER_CORE],
            "idx16": _make_idx16(labels_np[lo:lo + ROWS_PER_CORE]),
            "centers": centers,
        })
    return in_maps


def run(x, labels, centers, **spmd_kwargs):
    """Run on the 8 NeuronCores; returns (loss, BassKernelResults)."""
    nc = _get_nc()
    in_maps = make_in_maps(x, labels, centers)
    res = run_bass_kernel_spmd(nc, in_maps, core_ids=list(range(N_CORES)), **spmd_kwargs)
    total = sum(float(r["out"][0, 0]) for r in res.results)
    return np.float32(total / N), res


def kernel(x, labels, centers):
    loss, _ = run(x, labels, centers)
    return loss

